# revision 26
# baseline (speedup 1.0000x reference)
"""Trainium2 Bass kernel for nn_AutoAttention_Layer (sparse_attention).

Math (folded from the reference):
  qbar[b,d] = sum_f fs[f] * q[b,f,d]
  u[b,:]    = (qbar[b,:] @ W_query) @ W_key.T
  score[b,t]= sum_d k[b,t,d] * u[b,d] + bias*D
  s         = sigmoid(score);  w = exp(s/8) ~= (1 + s/16)^2  (s/8 in (0,1/8))
  e         = w * mask  (mask = 1 if t < len else 1e-30; tiny keeps len=0 rows
              finite and reproduces the reference's uniform-softmax fallback)
  out[b,:]  = ((sum_t e*k) / sum_t e) @ W_value          # input v is unused

Engine split per 128-batch core (batch data parallel over 8 cores):
  PE   : score via 64 accumulating diag(u[:,d]) matmuls per t-chunk
  Pool : mask build, one diag group (affine_select)
  ACT  : k-chunk casts, diag replicas, e-broadcast replicas, sigmoid
  DVE  : qbar, u-chain PSUM hops, diag muls, exp-poly, e*k prod, tree-adds
  DMA  : constants, q, then k t-chunks; compute overlapped with k streaming
"""

import numpy as np

import concourse.bass as bass
from concourse import bacc
import concourse.mybir as mybir
from concourse.tile import TileContext
from concourse.bass_utils import run_bass_kernel_spmd

B, T, F, D = 1024, 200, 64, 64
NCORES = 8
BL = B // NCORES  # 128
F32 = mybir.dt.float32
BF16 = mybir.dt.bfloat16
I32 = mybir.dt.int32

# HW-safety knobs: exotic instructions that CoreSim accepts but may lack
# ucode/runtime support on this HW path. Flip individually to bisect.
USE_AFFINE = False        # gpsimd.affine_select for diag build
USE_POOL_COMPUTE = False  # gpsimd tensor ops beyond iota
USE_TTR = False           # DVE tensor_tensor_reduce fused op

TCS = [64, 48, 40, 32, 16]      # descending t-chunks: tail chain shrinks as
TOFF = [0, 64, 112, 152, 184]   # the serial DMA resource drains
# diag-group availability order (g2 on DVE lands first, then Pool's g0, ...)
DORDER = ([32 + i for i in range(16)] + [i for i in range(16)]
          + [16 + i for i in range(16)] + [48 + i for i in range(16)])


def _bc(ap, dims, off=0):
    """View an SBUF/DRAM AP with explicit free dims [[step, count], ...]."""
    return bass.AP(tensor=ap.tensor, offset=ap.offset + off, ap=[ap.ap[0]] + dims)


def build_nc(repeat=1, variant="full"):
    nc = bacc.Bacc()
    alu = mybir.AluOpType
    act = mybir.ActivationFunctionType

    q_d = nc.declare_dram_parameter("q", [BL, F * D], F32, isOutput=False)
    k_d = nc.declare_dram_parameter("k", [BL, T * D], F32, isOutput=False)
    kes_d = nc.declare_dram_parameter("kes", [BL, 1], I32, isOutput=False)
    fs_d = nc.declare_dram_parameter("fs", [F, 1], F32, isOutput=False)
    bias_d = nc.declare_dram_parameter("bias", [1, 1], F32, isOutput=False)
    wq_d = nc.declare_dram_parameter("wq", [D, D], F32, isOutput=False)
    wk_d = nc.declare_dram_parameter("wk", [D, D], F32, isOutput=False)
    wv_d = nc.declare_dram_parameter("wv", [D, D], F32, isOutput=False)
    out_d = nc.declare_dram_parameter("out", [BL, D], F32, isOutput=True)

    with TileContext(nc) as tc:
        with (
            tc.tile_pool(name="big", bufs=1) as big,
            tc.tile_pool(name="work", bufs=2) as work,
            tc.tile_pool(name="small", bufs=1) as small,
            tc.tile_pool(name="psum", bufs=1, space="PSUM") as psum,
            tc.tile_pool(name="psum2", bufs=2, space="PSUM") as psum2,
        ):
            # ---- input DMAs -----------------------------------------------
            # The DMA engine pool is one serial resource: order transfers by
            # when their consumers need them. fs first (gates qbar), then q,
            # then the W matrices (u-chain), then the k t-chunks.
            fs_b = small.tile([BL, F], F32)
            nc.sync.dma_start(
                out=fs_b,
                in_=bass.AP(tensor=fs_d[:, :].tensor, offset=fs_d[:, :].offset,
                            ap=[[0, BL], [1, F]]),
            )
            q_s = big.tile([BL, F * D], F32, tag="q_s")
            for c in range(4):
                nc.sync.dma_start(
                    out=q_s[:, c * 1024:(c + 1) * 1024],
                    in_=q_d[:, c * 1024:(c + 1) * 1024],
                )
            wq_s = small.tile([D, D], F32)
            nc.sync.dma_start(out=wq_s, in_=wq_d[:, :])
            wk_s = small.tile([D, D], F32)
            nc.sync.dma_start(out=wk_s, in_=wk_d[:, :])
            wv_s = small.tile([D, D], F32)
            nc.sync.dma_start(out=wv_s, in_=wv_d[:, :])
            k_s = big.tile([BL, T * D], F32, tag="k_s")
            for c in range(len(TCS)):
                lo, hi = TOFF[c] * D, (TOFF[c] + TCS[c]) * D
                nc.sync.dma_start(out=k_s[:, lo:hi], in_=k_d[:, lo:hi])

            bias_b = small.tile([BL, 1], F32)
            nc.gpsimd.dma_start(
                out=bias_b,
                in_=bass.AP(tensor=bias_d[:, :].tensor, offset=bias_d[:, :].offset,
                            ap=[[0, BL], [1, 1]]),
            )
            kes_s = small.tile([BL, 1], I32)
            nc.gpsimd.dma_start(out=kes_s, in_=kes_d[:, :])

            # ---- Pool: identities + sequence mask -------------------------
            pe_ = nc.gpsimd if USE_POOL_COMPUTE else nc.vector
            ident_i = small.tile([128, 128], I32)
            nc.gpsimd.iota(ident_i, [[1, 128]], base=0, channel_multiplier=-1)
            identf = small.tile([128, 128], F32)
            pe_.tensor_scalar(
                out=identf, in0=ident_i, scalar1=0, scalar2=None, op0=alu.is_equal
            )
            identb = small.tile([128, 128], BF16)
            pe_.tensor_scalar(
                out=identb, in0=ident_i, scalar1=0, scalar2=None, op0=alu.is_equal
            )
            iota_i = small.tile([BL, T], I32)
            nc.gpsimd.iota(iota_i, [[1, T]], base=0, channel_multiplier=0)
            iota_f = small.tile([BL, T], F32)
            pe_.tensor_copy(out=iota_f, in_=iota_i)
            len_f = small.tile([BL, 1], F32)
            pe_.tensor_copy(out=len_f, in_=kes_s)
            mask01 = small.tile([BL, T], F32)
            pe_.tensor_scalar(
                out=mask01, in0=iota_f, scalar1=len_f[:], scalar2=None, op0=alu.is_lt
            )
            # mask_bf = 1.0 where valid, 1e-30 where masked (len=0 fallback)
            mask_bf = small.tile([BL, T], BF16)
            pe_.tensor_scalar(
                out=mask_bf, in0=mask01, scalar1=1.0, scalar2=1e-30,
                op0=alu.mult, op1=alu.add,
            )

            # ACT: pin the sigmoid act-func table before any Copy activation
            # so only one table load happens (Copy is in every table).
            dum = small.tile([BL, 1], F32)
            nc.scalar.activation(dum, bias_b, act.Sigmoid, bias=0.0, scale=1.0)

            # k chunk-0 cast early on ACT (k arrives ~10us; ACT idle then)
            k_bf = big.tile([BL, T * D], BF16, tag="k_bf")
            nc.scalar.copy(out=k_bf[:, 0:TCS[0] * D], in_=k_s[:, 0:TCS[0] * D])

            # ---- qbar = sum_f fs[f]*q  (ACT casts, DVE 2x TTs + trees) ----
            fs_exp = big.tile([BL, F * D], BF16, tag="fs_exp")
            nc.scalar.copy(out=fs_exp, in_=_bc(fs_b[:], [[1, F], [0, D]]))
            prod_q = big.tile([BL, F * D], BF16, tag="prod_q")
            for c in range(4):
                o = c * 1024
                nc.vector.tensor_tensor(
                    out=prod_q[:, o:o + 1024],
                    in0=q_s[:, o:o + 1024],
                    in1=fs_exp[:, o:o + 1024],
                    op=alu.mult,
                )
            qtails = []
            for c in range(4):
                o = c * 1024
                tq1 = work.tile([BL, 512], BF16, tag="tq1")
                nc.vector.tensor_tensor(out=tq1, in0=prod_q[:, o:o + 512],
                                        in1=prod_q[:, o + 512:o + 1024], op=alu.add)
                tq2 = work.tile([BL, 256], BF16, tag="tq2")
                nc.vector.tensor_tensor(out=tq2, in0=tq1[:, :256],
                                        in1=tq1[:, 256:512], op=alu.add)
                tq3 = work.tile([BL, 128], BF16, tag="tq3")
                nc.vector.tensor_tensor(out=tq3, in0=tq2[:, :128],
                                        in1=tq2[:, 128:256], op=alu.add)
                qtails.append(tq3)
            tq4a = work.tile([BL, 128], BF16, tag="tq4")
            nc.vector.tensor_tensor(out=tq4a, in0=qtails[0], in1=qtails[1],
                                    op=alu.add)
            tq4b = work.tile([BL, 128], BF16, tag="tq4")
            nc.vector.tensor_tensor(out=tq4b, in0=qtails[2], in1=qtails[3],
                                    op=alu.add)
            tq5 = work.tile([BL, 128], BF16, tag="tq5")
            nc.vector.tensor_tensor(out=tq5, in0=tq4a, in1=tq4b, op=alu.add)
            qbar = small.tile([BL, D], F32)
            nc.vector.tensor_reduce(
                out=qbar, in_=_bc(tq5[:], [[1, D], [D, 2]]),
                axis=mybir.AxisListType.X, op=alu.add,
            )

            # ---- M = Wq @ Wk.T precomputed off the qbar critical path -----
            wqT_p = psum.tile([D, D], F32, tag="ps_wkT")
            nc.tensor.transpose(wqT_p, wq_s, identf[:D, :D])
            wqT = small.tile([D, D], F32)
            nc.vector.tensor_copy(out=wqT, in_=wqT_p)
            wkT_p = psum2.tile([D, BL], F32, tag="ps_t")
            nc.tensor.transpose(wkT_p[:, :D], wk_s, identf[:D, :D])
            wkT = small.tile([D, D], F32)
            nc.vector.tensor_copy(out=wkT, in_=wkT_p[:, :D])
            m_p = psum2.tile([D, BL], F32, tag="ps_t")
            nc.tensor.matmul(m_p[:, :D], wqT, wkT, start=True, stop=True)
            m_s = small.tile([D, D], F32)
            nc.vector.tensor_copy(out=m_s, in_=m_p[:, :D])

            # ---- u^T = M^T @ qbar^T  (3 hops after qbar) ------------------
            # (priority-pinned: this short chain gates the diag build)
            qbarT_p = psum2.tile([D, BL], F32, tag="ps_t")
            nc.tensor.transpose(qbarT_p, qbar, identf)
            qbarT = small.tile([D, BL], F32)
            nc.vector.tensor_copy(out=qbarT, in_=qbarT_p)

            u_p = psum2.tile([BL, D], F32, tag="ps_v")
            nc.tensor.matmul(u_p, qbarT, m_s, start=True, stop=True)
            u_bf = small.tile([BL, D], BF16)
            nc.vector.tensor_copy(out=u_bf, in_=u_p)

            # ---- diag(u[:,d]) weights, split across DVE/ACT/Pool ----------
            GW = 16 * 128
            diag = big.tile([BL, D * 128], BF16, tag="diag")
            with tc.high_priority():
                # g2 fully on DVE (earliest available)
                urep2 = work.tile([BL, GW], BF16, tag="urep2", bufs=1)
                nc.vector.tensor_copy(out=urep2,
                                      in_=_bc(u_bf[:], [[1, 16], [0, 128]], off=32))
                nc.vector.tensor_tensor(
                    out=diag[:, 2 * GW:3 * GW],
                    in0=_bc(identb[:], [[0, 16], [1, 128]]),
                    in1=urep2, op=alu.mult)
                # g0 on Pool (affine) or via ACT repl + DVE mult
                if USE_AFFINE:
                    nc.gpsimd.affine_select(
                        out=diag[:, 0:GW],
                        in_=_bc(u_bf[:], [[1, 16], [0, 128]], off=0),
                        pattern=[[0, 16], [1, 128]],
                        compare_op=alu.is_equal,
                        fill=0.0,
                        base=0,
                        channel_multiplier=-1,
                    )
                    G_ACT = (1, 3)
                else:
                    G_ACT = (0, 1, 3)
                # ACT replicates, DVE multiplies by identity
                for g in G_ACT:
                    urep = work.tile([BL, GW], BF16, tag="urep")
                    nc.scalar.copy(out=urep,
                                   in_=_bc(u_bf[:], [[1, 16], [0, 128]], off=g * 16))
                    nc.vector.tensor_tensor(
                        out=diag[:, g * GW:(g + 1) * GW],
                        in0=_bc(identb[:], [[0, 16], [1, 128]]),
                        in1=urep, op=alu.mult)

            # bias*D for the sigmoid bias operand
            bias64 = small.tile([BL, 1], F32)
            nc.vector.tensor_scalar_mul(bias64, bias_b, float(D))

            # ---- per t-chunk, pass 1a: cast, PE score, sigmoid ------------
            ps_score = psum.tile([BL, T], F32, tag="ps_score")
            CAST_ENG = [None, "dve", "act", "act", "dve"]
            EEXP_ENG = ["dve", "act", "act", "act", None]
            sig_cs = []
            for c, tcsz in enumerate(TCS):
                toff = TOFF[c]
                lo, hi = toff * D, (toff + tcsz) * D
                if CAST_ENG[c] == "act":
                    nc.scalar.copy(out=k_bf[:, lo:hi], in_=k_s[:, lo:hi])
                elif CAST_ENG[c] == "dve":
                    nc.vector.tensor_copy(out=k_bf[:, lo:hi], in_=k_s[:, lo:hi])

                # PE: score[:, chunk] = sum_d diag_d @ k_bf[:, d::D]
                # (d in diag-group availability order; accumulation commutes)
                for i, d in enumerate(DORDER):
                    nc.tensor.matmul(
                        ps_score[:, toff:toff + tcsz],
                        _bc(diag[:], [[1, 128]], off=d * 128),
                        _bc(k_bf[:], [[D, tcsz]], off=lo + d),
                        start=(i == 0), stop=(i == D - 1),
                    )
                # sigmoid(score + bias*D) straight out of PSUM
                sig_c = work.tile([BL, tcsz], BF16, tag="sig")
                nc.scalar.activation(sig_c, ps_score[:, toff:toff + tcsz],
                                     act.Sigmoid, bias=bias64[:], scale=1.0)
                sig_cs.append(sig_c)

            # ---- pass 1b: w = (1+sig/16)^2, e = w*mask, se chain, e_exp ---
            se_prev = None
            e_cs, e_exps = [], []
            for c, tcsz in enumerate(TCS):
                toff = TOFF[c]
                t_c = work.tile([BL, tcsz], BF16, tag="tpoly")
                nc.vector.tensor_scalar(
                    out=t_c, in0=sig_cs[c], scalar1=1.0 / 16.0, scalar2=1.0,
                    op0=alu.mult, op1=alu.add,
                )
                tm_c = work.tile([BL, tcsz], BF16, tag="tmpoly")
                nc.vector.tensor_tensor(
                    out=tm_c, in0=t_c, in1=mask_bf[:, toff:toff + tcsz],
                    op=alu.mult,
                )
                e_c = work.tile([BL, tcsz], BF16, tag="e")
                se_c = work.tile([BL, 1], F32, tag="se")
                if USE_TTR:
                    nc.vector.tensor_tensor_reduce(
                        out=e_c, in0=t_c, in1=tm_c, scale=1.0,
                        scalar=(0.0 if se_prev is None else se_prev[:]),
                        op0=alu.mult, op1=alu.add, accum_out=se_c[:],
                    )
                else:
                    nc.vector.tensor_tensor(out=e_c, in0=t_c, in1=tm_c,
                                            op=alu.mult)
                    sp_c = work.tile([BL, 1], F32, tag="sep")
                    nc.vector.tensor_reduce(out=sp_c, in_=e_c,
                                            axis=mybir.AxisListType.X, op=alu.add)
                    if se_prev is None:
                        se_c = sp_c
                    else:
                        nc.vector.tensor_tensor(out=se_c, in0=sp_c,
                                                in1=se_prev, op=alu.add)
                se_prev = se_c
                e_cs.append(e_c)
                if EEXP_ENG[c] == "act":
                    e_exp = work.tile([BL, tcsz * D], BF16, tag="eexp")
                    nc.scalar.copy(out=e_exp,
                                   in_=_bc(e_c[:], [[1, tcsz], [0, D]]))
                elif EEXP_ENG[c] == "dve":
                    e_exp = work.tile([BL, tcsz * D], BF16, tag="eexp")
                    nc.vector.tensor_copy(out=e_exp,
                                          in_=_bc(e_c[:], [[1, tcsz], [0, D]]))
                else:
                    e_exp = None
                e_exps.append(e_exp)

            # ---- per t-chunk, pass 2: prod = k*e, halving tree over t -----
            abar_parts = []
            for c, tcsz in enumerate(TCS):
                toff = TOFF[c]
                lo, hi = toff * D, (toff + tcsz) * D
                prod_c = work.tile([BL, tcsz * D], BF16, tag="prod")
                if e_exps[c] is not None:
                    nc.vector.tensor_tensor(
                        out=prod_c, in0=k_bf[:, lo:hi], in1=e_exps[c],
                        op=alu.mult,
                    )
                else:
                    nc.vector.tensor_tensor(
                        out=prod_c, in0=k_bf[:, lo:hi],
                        in1=_bc(e_cs[c][:], [[1, tcsz], [0, D]]), op=alu.mult,
                    )
                # halving tree inside one scratch tile (levels at offsets)
                scratch = work.tile([BL, tcsz * D], BF16, tag="tree")
                cur, off, w = prod_c, 0, tcsz * D
                while w > 2 * D and (w // D) % 2 == 0:
                    w //= 2
                    nxt = scratch[:, off:off + w]
                    eng = (nc.gpsimd if (USE_POOL_COMPUTE and w <= 512 and c < 3)
                           else nc.vector)
                    eng.tensor_tensor(out=nxt, in0=cur[:, :w],
                                      in1=cur[:, w:2 * w], op=alu.add)
                    cur = nxt
                    off += w
                abar_c = work.tile([BL, D], F32, tag="abar_c")
                if w == 2 * D:
                    nc.vector.tensor_tensor(out=abar_c, in0=cur[:, :D],
                                            in1=cur[:, D:2 * D], op=alu.add)
                else:
                    nc.vector.tensor_reduce(
                        out=abar_c, in_=_bc(cur[:], [[1, D], [D, w // D]]),
                        axis=mybir.AxisListType.X, op=alu.add,
                    )
                abar_parts.append(abar_c)

            ab01 = work.tile([BL, D], F32, tag="ab01")
            nc.vector.tensor_tensor(out=ab01, in0=abar_parts[0],
                                    in1=abar_parts[1], op=alu.add)
            ab23 = work.tile([BL, D], F32, tag="ab23")
            nc.vector.tensor_tensor(out=ab23, in0=abar_parts[2],
                                    in1=abar_parts[3], op=alu.add)
            ab014 = work.tile([BL, D], F32, tag="ab014")
            nc.vector.tensor_tensor(out=ab014, in0=ab01,
                                    in1=abar_parts[4], op=alu.add)
            rs = small.tile([BL, 1], F32)
            nc.vector.reciprocal(rs, se_prev)
            abar = small.tile([BL, D], F32)
            nc.vector.tensor_tensor(out=abar, in0=ab014, in1=ab23, op=alu.add)
            nc.vector.tensor_scalar(
                out=abar, in0=abar, scalar1=rs[:], scalar2=None, op0=alu.mult
            )

            # ---- out = abar @ W_value  (f32 PE chain) ---------------------
            abarT_p = psum2.tile([D, BL], F32, tag="ps_t")
            nc.tensor.transpose(abarT_p, abar, identf)
            abarT = small.tile([D, BL], F32)
            nc.scalar.copy(out=abarT, in_=abarT_p)

            o_p = psum2.tile([BL, D], F32, tag="ps_v")
            nc.tensor.matmul(o_p, abarT, wv_s, start=True, stop=True)
            o_s = small.tile([BL, D], F32)
            nc.scalar.copy(out=o_s, in_=o_p)
            nc.sync.dma_start(out=out_d[:, :], in_=o_s)

    nc.finalize()
    return nc


_NC_CACHE = {}


def _get_nc():
    if "nc" not in _NC_CACHE:
        _NC_CACHE["nc"] = build_nc()
    return _NC_CACHE["nc"]


def make_in_maps(q, k, kes_length, field_strengths, bias, W_query, W_key, W_value):
    f32 = np.float32
    maps = []
    for c in range(NCORES):
        sl = slice(c * BL, (c + 1) * BL)
        maps.append({
            "q": np.ascontiguousarray(q[sl].reshape(BL, F * D), dtype=f32),
            "k": np.ascontiguousarray(k[sl].reshape(BL, T * D), dtype=f32),
            "kes": np.ascontiguousarray(kes_length[sl].reshape(BL, 1), dtype=np.int32),
            "fs": np.ascontiguousarray(field_strengths.reshape(F, 1), dtype=f32),
            "bias": np.ascontiguousarray(bias.reshape(1, 1), dtype=f32),
            "wq": np.ascontiguousarray(W_query, dtype=f32),
            "wk": np.ascontiguousarray(W_key, dtype=f32),
            "wv": np.ascontiguousarray(W_value, dtype=f32),
        })
    return maps


def kernel(q, k, v, kes_length, field_strengths, bias, W_query, W_key, W_value,
           **_unused):
    nc = _get_nc()
    in_maps = make_in_maps(np.asarray(q), np.asarray(k), np.asarray(kes_length),
                           np.asarray(field_strengths), np.asarray(bias),
                           np.asarray(W_query), np.asarray(W_key),
                           np.asarray(W_value))
    res = run_bass_kernel_spmd(nc, in_maps, list(range(NCORES)))
    out = np.concatenate([res.results[c]["out"] for c in range(NCORES)], axis=0)
    return out.reshape(B, 1, D).astype(np.float32)
